# revision 18
# baseline (speedup 1.0000x reference)
"""Trainium2 Bass kernel for nn_Beltrami (retrieval_knn).

Per-core (batch-parallel over 8 cores): fc (f32r hi/lo-split matmuls for exact
pos; hi-only for feat and row norms) -> normalize pos -> quantize posT to fp16
-> cosine sim via 2-pass fp16 matmul (sim noise ~2.5e-5; measured end-to-end
rel err 1.27e-2 < 2e-2 gate) -> exp on Act (PSUM->SBUF, E32 f32, doubles as
the PSUM-freeing copy) -> top-32 threshold via 16x max8 over 256-col chunks +
4-round max8/match_replace refine in exp space -> fused (E>=v32)*E mask via
scalar_tensor_tensor with accum_out denominator -> SBUF->SBUF DMA transpose of
A -> A@feat fp16 gather matmul (software-pipelined 3 tiles behind the sim so
the PE stream stays dense) -> softmax-normalized out.
"""
import sys
import numpy as np

sys.path.insert(0, "/opt/trn_rl_repo")

B, N, C, K = 8, 4096, 256, 32
NT = N // 128          # 32 query tiles of 128 rows
NEG = -1.0e30

_CACHE = {}


def _build(reps=1):
    from contextlib import ExitStack
    import concourse.bass as bass
    import concourse.bacc as bacc
    import concourse.tile as tile
    from concourse import mybir

    f32 = mybir.dt.float32
    f32r = mybir.dt.float32r
    f16 = mybir.dt.float16
    AF = mybir.ActivationFunctionType
    Alu = mybir.AluOpType

    nc = bacc.Bacc("TRN2", target_bir_lowering=False, debug=False, num_devices=8)

    xT_in = nc.declare_dram_parameter("xT", [C, N], f32, isOutput=False)
    wT_in = nc.declare_dram_parameter("wT", [C, 2 * C], f32, isOutput=False)
    bf_in = nc.declare_dram_parameter("bf", [1, 2 * C], f32, isOutput=False)
    bp_in = nc.declare_dram_parameter("bp", [128, 2], f32, isOutput=False)
    id_in = nc.declare_dram_parameter("ident", [128, 128], f32, isOutput=False)
    out_p = nc.declare_dram_parameter("out", [N, C], f32, isOutput=True)
    s_dram = nc.dram_tensor("s_scratch", [NT, 128], f32)

    with tile.TileContext(nc) as tc, ExitStack() as ctx:
        # ---------------- persistent pools ----------------
        persist = ctx.enter_context(tc.tile_pool(name="persist", bufs=1))
        featx_pool = ctx.enter_context(tc.tile_pool(name="featx", bufs=NT))

        # fp16 normalized posT, the only sim operand kept resident
        post16 = [persist.tile([128, N], f16, tag=f"post16_{ct}", name=f"post16_{ct}")
                  for ct in range(2)]
        featx = [featx_pool.tile([128, C + 2], f16, tag="featx", name=f"featx{i}") for i in range(NT)]
        nrm2 = persist.tile([128, NT], f32, tag="nrm2")
        s_til = persist.tile([128, NT], f32, tag="s_til")

        # ---------------- startup: fc + normalize ----------------
        with ExitStack() as sctx:
            sb = sctx.enter_context(tc.tile_pool(name="start_sb", bufs=1))
            ps_fc = sctx.enter_context(tc.tile_pool(name="ps_fc", bufs=3, space="PSUM"))
            ps_pp = sctx.enter_context(tc.tile_pool(name="ps_pp", bufs=3, space="PSUM"))
            ps_tp = sctx.enter_context(tc.tile_pool(name="ps_tp", bufs=1, space="PSUM"))

            xstage_pool = sctx.enter_context(tc.tile_pool(name="xstage_pool", bufs=3))
            xt_r = [sb.tile([128, N], f32r, tag=f"xt_r{ct}", name=f"xt_r{ct}") for ct in range(2)]
            wt = [sb.tile([128, 2 * C], f32, tag=f"wt{ct}", name=f"wt{ct}") for ct in range(2)]
            wt_r = [sb.tile([128, C], f32r, tag=f"wt_r{ct}", name=f"wt_r{ct}") for ct in range(2)]
            xt_lo = [sb.tile([128, N], f32r, tag=f"xt_lo{ct}", name=f"xt_lo{ct}") for ct in range(2)]
            wph = [sb.tile([128, C], f32r, tag=f"wph{ct}", name=f"wph{ct}") for ct in range(2)]
            wpl = [sb.tile([128, C], f32r, tag=f"wpl{ct}", name=f"wpl{ct}") for ct in range(2)]
            bf1_r = sb.tile([1, 2 * C], f32r, tag="bf1_r")
            ones1_r = sb.tile([1, 128], f32r, tag="ones1_r")
            bf1 = sb.tile([1, 2 * C], f32, tag="bf1")
            bp = sb.tile([128, 2], f32, tag="bp")
            ident = sb.tile([128, 128], f32, tag="ident")
            ones1 = sb.tile([1, 128], f32, tag="ones1")
            scrap = sb.tile([128, C], f16, tag="scrap")
            post_raw = [sb.tile([128, N], f32, tag=f"post_raw{ct}", name=f"post_raw{ct}") for ct in range(2)]

            for ct in range(2):
                nc.sync.dma_start(wt[ct][:], wT_in[ct * 128:(ct + 1) * 128, :])
            nc.sync.dma_start(bf1[:], bf_in[:])
            nc.sync.dma_start(bp[:], bp_in[:])
            nc.sync.dma_start(ident[:], id_in[:])
            nc.vector.memset(ones1[:], 1.0)
            for ct in range(2):
                nc.vector.tensor_copy(wt_r[ct][:], wt[ct][:, 0:C])
                nc.vector.tensor_copy(wph[ct][:], wt[ct][:, C:2 * C])
                nc.vector.tensor_tensor(wpl[ct][:], wt[ct][:, C:2 * C],
                                        wph[ct][:], op=Alu.subtract)
            nc.vector.tensor_copy(bf1_r[:], bf1[:])
            nc.vector.tensor_copy(ones1_r[:], ones1[:])

            # staging + fc interleaved per 512-chunk so PE stays dense
            def stage_chunk(ch):
                cs = slice(ch * 512, (ch + 1) * 512)
                for ct in range(2):
                    xstage = xstage_pool.tile([128, 512], f32, tag="xstage",
                                              name=f"xstage{ct}_{ch}")
                    nc.sync.dma_start(xstage[:], xT_in[ct * 128:(ct + 1) * 128, cs])
                    nc.vector.tensor_copy(xt_r[ct][:, cs], xstage[:])
                    nc.vector.tensor_tensor(xt_lo[ct][:, cs], xstage[:],
                                            xt_r[ct][:, cs], op=Alu.subtract)

            def posT_chunk(dt, ch):
                pp = ps_pp.tile([128, 512], f32, tag="pp", name=f"pp{dt}_{ch}")
                ds_ = slice(dt * 128, (dt + 1) * 128)
                cs_ = slice(ch * 512, (ch + 1) * 512)
                for ci, (lh, rh) in enumerate(
                        [(wph[0], xt_r[0]), (wph[0], xt_lo[0]), (wpl[0], xt_r[0]),
                         (wph[1], xt_r[1]), (wph[1], xt_lo[1]), (wpl[1], xt_r[1])]):
                    nc.tensor.matmul(pp[:], lh[:, ds_], rh[:, cs_],
                                     start=(ci == 0), stop=(ci == 5))
                nc.scalar.activation(
                    post_raw[dt][:, ch * 512:(ch + 1) * 512], pp[:],
                    AF.Identity, bias=bp[:, dt:dt + 1])

            for ch in range(8):
                stage_chunk(ch)
            for nt in range(NT):
                fc = ps_fc.tile([128, 2 * C], f32, tag="fc")
                ns = slice(nt * 128, (nt + 1) * 128)
                nc.tensor.matmul(fc[:, 0:C], xt_r[0][:, ns], wt_r[0][:],
                                 start=True, stop=False)
                nc.tensor.matmul(fc[:, 0:C], xt_r[1][:, ns], wt_r[1][:],
                                 start=False, stop=False)
                nc.tensor.matmul(fc[:, 0:C], ones1_r[:], bf1_r[:, 0:C],
                                 start=False, stop=True)
                # hi-only pos fc: nrm2 tolerates ~1e-3 relative error (a
                # norm error scales a whole sim row, never reordering it)
                nc.tensor.matmul(fc[:, C:2 * C], xt_r[0][:, ns], wph[0][:],
                                 start=True, stop=False)
                nc.tensor.matmul(fc[:, C:2 * C], xt_r[1][:, ns], wph[1][:],
                                 start=False, stop=False)
                nc.tensor.matmul(fc[:, C:2 * C], ones1_r[:], bf1_r[:, C:2 * C],
                                 start=False, stop=True)
                nc.scalar.activation(featx[nt][:, 0:C], fc[:, 0:C], AF.Copy)
                nc.gpsimd.memset(featx[nt][:, C:C + 1], 1.0)
                nc.gpsimd.memset(featx[nt][:, C + 1:C + 2], 0.0)
                nc.scalar.activation(scrap[:], fc[:, C:2 * C], AF.Square,
                                     accum_out=nrm2[:, nt:nt + 1])

            # rsqrt of norms with two Newton steps (overlaps posT below)
            r0 = sb.tile([128, NT], f32, tag="r0")
            u = sb.tile([128, NT], f32, tag="u")
            nc.vector.reciprocal(r0[:], nrm2[:])
            nc.scalar.activation(s_til[:], r0[:], AF.Sqrt)
            for _ in range(2):
                nc.vector.tensor_tensor(u[:], s_til[:], s_til[:], op=Alu.mult)
                nc.vector.tensor_tensor(u[:], u[:], nrm2[:], op=Alu.mult)
                nc.vector.tensor_scalar(u[:], u[:], -0.5, scalar2=1.5,
                                        op0=Alu.mult, op1=Alu.add)
                nc.vector.tensor_tensor(s_til[:], s_til[:], u[:], op=Alu.mult)

            # transpose s [128, NT] -> [NT, 128], bounce via DRAM, broadcast-load
            st_ps = ps_tp.tile([NT, 128], f32, tag="st_ps")
            nc.tensor.transpose(st_ps[:], s_til[:], ident[:])
            stt = sb.tile([NT, 128], f32, tag="stt")
            nc.vector.tensor_copy(stt[:], st_ps[:])
            nc.sync.dma_start(s_dram[:], stt[:])

            # posT + scale-to-f16 streamed per chunk
            for ch in range(8):
                posT_chunk(0, ch)
                posT_chunk(1, ch)
                cs = slice(ch * 512, (ch + 1) * 512)
                sbc = xstage_pool.tile([128, 512], f32, tag="sbc",
                                       name=f"sbc{ch}")
                nc.sync.dma_start(
                    sbc[:], s_dram[:].flatten()[cs].partition_broadcast(128))
                for ct in range(2):
                    nc.vector.tensor_tensor(post16[ct][:, cs],
                                            post_raw[ct][:, cs],
                                            sbc[:], op=Alu.mult)

        # ---------------- steady loop over query tiles ----------------
        e_pool = ctx.enter_context(tc.tile_pool(name="e_sb", bufs=2))
        a_pool = ctx.enter_context(tc.tile_pool(name="a_sb", bufs=2))
        m_pool = ctx.enter_context(tc.tile_pool(name="m_sb", bufs=2))
        at_pool = ctx.enter_context(tc.tile_pool(name="at_sb", bufs=5))
        osb_pool = ctx.enter_context(tc.tile_pool(name="osb_sb", bufs=2))
        cands_pool = ctx.enter_context(tc.tile_pool(name="cands_sb", bufs=2))
        asum_pool = ctx.enter_context(tc.tile_pool(name="asum_sb", bufs=6))
        ps_sim = ctx.enter_context(tc.tile_pool(name="ps_sim", bufs=5, space="PSUM"))
        ps_oe = ctx.enter_context(tc.tile_pool(name="ps_oe", bufs=2, space="PSUM"))

        PIPE = 3  # gather matmuls lag the sim by 3 tiles to hide selection+DMA

        front = {}   # T -> (E32, cands)
        state = {}   # T -> (AT, asum, oe)

        def emit_front_half(T, half):
            qs = slice(T * 128, (T + 1) * 128)
            if half == 0:
                E32 = e_pool.tile([128, N], f32, tag="E32", name=f"E32_{T}")
                cands = cands_pool.tile([128, 128], f32, tag="cands")
                front[T] = (E32, cands)
            E32, cands = front[T]
            # 2-pass fp16 sim (stationary shared across 4 moving chunks to
            # coalesce weight loads), exp to SBUF, max8 cands per 256 cols
            sms = [ps_sim.tile([128, 512], f32, tag="sm",
                               name=f"sm{T}_{half}_{r}") for r in range(4)]
            for ct in range(2):
                for r in range(4):
                    o = half * 2048 + r * 512
                    nc.tensor.matmul(sms[r][:], post16[ct][:, qs],
                                     post16[ct][:, o:o + 512],
                                     start=(ct == 0), stop=(ct == 1))
            for r in range(4):
                o = half * 2048 + r * 512
                nc.scalar.activation(E32[:, o:o + 512], sms[r][:], AF.Exp)
                if r % 2 == 1:
                    for c in range(4):
                        gc = (half * 2 + r // 2) * 4 + c
                        nc.vector.max(cands[:, gc * 8:(gc + 1) * 8],
                                      E32[:, gc * 256:(gc + 1) * 256])

        def emit_selection(T):
            E32, cands = front.pop(T)
            # exact top-32 threshold in exp space: 4 rounds of max8+replace
            r8 = cands_pool.tile([128, 8], f32, tag="r8")
            for rnd in range(4):
                nc.vector.max(r8[:], cands[:])
                if rnd < 3:
                    nc.vector.match_replace(out=cands[:], in_to_replace=r8[:],
                                            in_values=cands[:], imm_value=NEG)

            # mask on DVE at 2x, weight multiply on the idle Pool engine
            M16 = m_pool.tile([128, N], f16, tag="M16", name=f"M16_{T}")
            nc.vector.tensor_scalar(M16[:], E32[:], r8[:, 7:8], scalar2=None,
                                    op0=Alu.is_ge)
            A = a_pool.tile([128, N], f16, tag="A")
            nc.gpsimd.tensor_tensor(A[:], E32[:], M16[:], op=Alu.mult)

            # blocked transpose, SBUF -> SBUF (no DRAM bounce)
            AT = at_pool.tile([128, NT, 128], f16, tag="AT", name=f"AT_{T}")
            nc.sync.dma_start_transpose(AT[:], A[:])
            state[T] = (AT, None)

        def emit_back_half(T, half):
            AT, asum = state[T][:2]
            if half == 0:
                oe = ps_oe.tile([128, C + 2], f32, tag="oe", name=f"oe{T}")
                state[T] = (AT, asum, oe)
            else:
                oe = state[T][2]
            for j in range(half * 16, half * 16 + 16):
                nc.tensor.matmul(oe[:], AT[:, j, :], featx[j][:],
                                 start=(j == 0), stop=(j == NT - 1))

        def emit_back_fin(T):
            AT, asum, oe = state.pop(T)
            rz = asum_pool.tile([128, 1], f32, tag="rz")
            nc.vector.reciprocal(rz[:], oe[:, C:C + 1])
            osb = osb_pool.tile([128, C], f32, tag="osb")
            nc.scalar.activation(osb[:], oe[:, 0:C], AF.Copy, scale=rz[:])
            nc.sync.dma_start(out_p[T * 128:(T + 1) * 128, :], osb[:])

        for rep in range(reps):
            for T in range(NT + PIPE):
                # interleave gather halves of tile T-PIPE into tile T's sim
                # stream so the PE sees one dense instruction sequence
                if T < NT:
                    emit_front_half(T, 0)
                if T >= PIPE:
                    emit_back_half(T - PIPE, 0)
                if T < NT:
                    emit_front_half(T, 1)
                if T >= PIPE:
                    emit_back_half(T - PIPE, 1)
                if T < NT:
                    emit_selection(T)
                if T >= PIPE:
                    emit_back_fin(T - PIPE)

    nc.compile()
    return nc


def kernel(x, W, bias, k):
    from concourse.bass_utils import run_bass_kernel_spmd

    x = np.asarray(x, dtype=np.float32)
    W = np.asarray(W, dtype=np.float32)
    bias = np.asarray(bias, dtype=np.float32)
    assert int(k) == K and x.shape == (B, N, C)

    if "nc" not in _CACHE:
        _CACHE["nc"] = _build()
    nc = _CACHE["nc"]

    wT = np.ascontiguousarray(W.T)                      # [C, 2C]
    bf = bias.reshape(1, 2 * C)
    bp = np.ascontiguousarray(
        bias[C:].reshape(2, 128).T)                     # [128, 2]
    ident = np.eye(128, dtype=np.float32)

    in_maps = []
    for b in range(B):
        xT = np.ascontiguousarray(x[b].T)               # [C, N]
        in_maps.append({"xT": xT, "wT": wT, "bf": bf, "bp": bp, "ident": ident})

    res = run_bass_kernel_spmd(nc, in_maps, list(range(B)))
    out = np.stack([res.results[b]["out"] for b in range(B)], axis=0)
    return out.astype(np.float32)


# revision 19
# speedup vs baseline: 1.2238x; 1.2238x over previous
"""Trainium2 Bass kernel for nn_Beltrami (retrieval_knn).

Per-core (batch-parallel over 8 cores): fc (f32r hi/lo-split matmuls for exact
pos; hi-only for feat and row norms) -> normalize pos -> quantize posT to fp16
-> cosine sim via 2-pass fp16 matmul (sim noise ~2.5e-5; measured end-to-end
rel err 1.27e-2 < 2e-2 gate) -> exp on Act (PSUM->SBUF, E32 f32, doubles as
the PSUM-freeing copy) -> top-32 threshold via 16x max8 over 256-col chunks +
4-round max8/match_replace refine in exp space -> fused (E>=v32)*E mask via
scalar_tensor_tensor with accum_out denominator -> SBUF->SBUF DMA transpose of
A -> A@feat fp16 gather matmul (software-pipelined 3 tiles behind the sim so
the PE stream stays dense) -> softmax-normalized out.
"""
import sys
import numpy as np

sys.path.insert(0, "/opt/trn_rl_repo")

B, N, C, K = 8, 4096, 256, 32
NT = N // 128          # 32 query tiles of 128 rows
NEG = -1.0e30

_CACHE = {}


def _build(reps=1):
    from contextlib import ExitStack
    import concourse.bass as bass
    import concourse.bacc as bacc
    import concourse.tile as tile
    from concourse import mybir

    f32 = mybir.dt.float32
    f32r = mybir.dt.float32r
    f16 = mybir.dt.float16
    AF = mybir.ActivationFunctionType
    Alu = mybir.AluOpType

    nc = bacc.Bacc("TRN2", target_bir_lowering=False, debug=False, num_devices=8)

    xT_in = nc.declare_dram_parameter("xT", [C, N], f32, isOutput=False)
    wT_in = nc.declare_dram_parameter("wT", [C, 2 * C], f32, isOutput=False)
    bf_in = nc.declare_dram_parameter("bf", [1, 2 * C], f32, isOutput=False)
    bp_in = nc.declare_dram_parameter("bp", [128, 2], f32, isOutput=False)
    id_in = nc.declare_dram_parameter("ident", [128, 128], f32, isOutput=False)
    out_p = nc.declare_dram_parameter("out", [N, C], f32, isOutput=True)
    s_dram = nc.dram_tensor("s_scratch", [NT, 128], f32)

    with tile.TileContext(nc) as tc, ExitStack() as ctx:
        # ---------------- persistent pools ----------------
        persist = ctx.enter_context(tc.tile_pool(name="persist", bufs=1))
        featx_pool = ctx.enter_context(tc.tile_pool(name="featx", bufs=NT))

        # fp16 normalized posT, the only sim operand kept resident
        post16 = [persist.tile([128, N], f16, tag=f"post16_{ct}", name=f"post16_{ct}")
                  for ct in range(2)]
        featx = [featx_pool.tile([128, C], f16, tag="featx", name=f"featx{i}") for i in range(NT)]
        nrm2 = persist.tile([128, NT], f32, tag="nrm2")
        s_til = persist.tile([128, NT], f32, tag="s_til")

        # ---------------- startup: fc + normalize ----------------
        with ExitStack() as sctx:
            sb = sctx.enter_context(tc.tile_pool(name="start_sb", bufs=1))
            ps_fc = sctx.enter_context(tc.tile_pool(name="ps_fc", bufs=3, space="PSUM"))
            ps_pp = sctx.enter_context(tc.tile_pool(name="ps_pp", bufs=3, space="PSUM"))
            ps_tp = sctx.enter_context(tc.tile_pool(name="ps_tp", bufs=1, space="PSUM"))

            xstage_pool = sctx.enter_context(tc.tile_pool(name="xstage_pool", bufs=3))
            xt_r = [sb.tile([128, N], f32r, tag=f"xt_r{ct}", name=f"xt_r{ct}") for ct in range(2)]
            wt = [sb.tile([128, 2 * C], f32, tag=f"wt{ct}", name=f"wt{ct}") for ct in range(2)]
            wt_r = [sb.tile([128, C], f32r, tag=f"wt_r{ct}", name=f"wt_r{ct}") for ct in range(2)]
            xt_lo = [sb.tile([128, N], f32r, tag=f"xt_lo{ct}", name=f"xt_lo{ct}") for ct in range(2)]
            wph = [sb.tile([128, C], f32r, tag=f"wph{ct}", name=f"wph{ct}") for ct in range(2)]
            wpl = [sb.tile([128, C], f32r, tag=f"wpl{ct}", name=f"wpl{ct}") for ct in range(2)]
            bf1_r = sb.tile([1, 2 * C], f32r, tag="bf1_r")
            ones1_r = sb.tile([1, 128], f32r, tag="ones1_r")
            bf1 = sb.tile([1, 2 * C], f32, tag="bf1")
            bp = sb.tile([128, 2], f32, tag="bp")
            ident = sb.tile([128, 128], f32, tag="ident")
            ones1 = sb.tile([1, 128], f32, tag="ones1")
            scrap = sb.tile([128, C], f16, tag="scrap")
            post_raw = [sb.tile([128, N], f32, tag=f"post_raw{ct}", name=f"post_raw{ct}") for ct in range(2)]

            for ct in range(2):
                nc.sync.dma_start(wt[ct][:], wT_in[ct * 128:(ct + 1) * 128, :])
            nc.sync.dma_start(bf1[:], bf_in[:])
            nc.sync.dma_start(bp[:], bp_in[:])
            nc.sync.dma_start(ident[:], id_in[:])
            nc.vector.memset(ones1[:], 1.0)
            for ct in range(2):
                nc.vector.tensor_copy(wt_r[ct][:], wt[ct][:, 0:C])
                nc.vector.tensor_copy(wph[ct][:], wt[ct][:, C:2 * C])
                nc.vector.tensor_tensor(wpl[ct][:], wt[ct][:, C:2 * C],
                                        wph[ct][:], op=Alu.subtract)
            nc.vector.tensor_copy(bf1_r[:], bf1[:])
            nc.vector.tensor_copy(ones1_r[:], ones1[:])

            # staging + fc interleaved per 512-chunk so PE stays dense
            def stage_chunk(ch):
                cs = slice(ch * 512, (ch + 1) * 512)
                for ct in range(2):
                    xstage = xstage_pool.tile([128, 512], f32, tag="xstage",
                                              name=f"xstage{ct}_{ch}")
                    nc.sync.dma_start(xstage[:], xT_in[ct * 128:(ct + 1) * 128, cs])
                    nc.vector.tensor_copy(xt_r[ct][:, cs], xstage[:])
                    nc.vector.tensor_tensor(xt_lo[ct][:, cs], xstage[:],
                                            xt_r[ct][:, cs], op=Alu.subtract)

            def posT_chunk(dt, ch):
                pp = ps_pp.tile([128, 512], f32, tag="pp", name=f"pp{dt}_{ch}")
                ds_ = slice(dt * 128, (dt + 1) * 128)
                cs_ = slice(ch * 512, (ch + 1) * 512)
                for ci, (lh, rh) in enumerate(
                        [(wph[0], xt_r[0]), (wph[0], xt_lo[0]), (wpl[0], xt_r[0]),
                         (wph[1], xt_r[1]), (wph[1], xt_lo[1]), (wpl[1], xt_r[1])]):
                    nc.tensor.matmul(pp[:], lh[:, ds_], rh[:, cs_],
                                     start=(ci == 0), stop=(ci == 5))
                nc.scalar.activation(
                    post_raw[dt][:, ch * 512:(ch + 1) * 512], pp[:],
                    AF.Identity, bias=bp[:, dt:dt + 1])

            for ch in range(8):
                stage_chunk(ch)
            for nt in range(NT):
                fc = ps_fc.tile([128, 2 * C], f32, tag="fc")
                ns = slice(nt * 128, (nt + 1) * 128)
                nc.tensor.matmul(fc[:, 0:C], xt_r[0][:, ns], wt_r[0][:],
                                 start=True, stop=False)
                nc.tensor.matmul(fc[:, 0:C], xt_r[1][:, ns], wt_r[1][:],
                                 start=False, stop=False)
                nc.tensor.matmul(fc[:, 0:C], ones1_r[:], bf1_r[:, 0:C],
                                 start=False, stop=True)
                # hi-only pos fc: nrm2 tolerates ~1e-3 relative error (a
                # norm error scales a whole sim row, never reordering it)
                nc.tensor.matmul(fc[:, C:2 * C], xt_r[0][:, ns], wph[0][:],
                                 start=True, stop=False)
                nc.tensor.matmul(fc[:, C:2 * C], xt_r[1][:, ns], wph[1][:],
                                 start=False, stop=False)
                nc.tensor.matmul(fc[:, C:2 * C], ones1_r[:], bf1_r[:, C:2 * C],
                                 start=False, stop=True)
                nc.scalar.activation(featx[nt][:], fc[:, 0:C], AF.Copy)
                nc.scalar.activation(scrap[:], fc[:, C:2 * C], AF.Square,
                                     accum_out=nrm2[:, nt:nt + 1])

            # rsqrt of norms with two Newton steps (overlaps posT below)
            r0 = sb.tile([128, NT], f32, tag="r0")
            u = sb.tile([128, NT], f32, tag="u")
            nc.vector.reciprocal(r0[:], nrm2[:])
            nc.scalar.activation(s_til[:], r0[:], AF.Sqrt)
            for _ in range(2):
                nc.vector.tensor_tensor(u[:], s_til[:], s_til[:], op=Alu.mult)
                nc.vector.tensor_tensor(u[:], u[:], nrm2[:], op=Alu.mult)
                nc.vector.tensor_scalar(u[:], u[:], -0.5, scalar2=1.5,
                                        op0=Alu.mult, op1=Alu.add)
                nc.vector.tensor_tensor(s_til[:], s_til[:], u[:], op=Alu.mult)

            # transpose s [128, NT] -> [NT, 128], bounce via DRAM, broadcast-load
            st_ps = ps_tp.tile([NT, 128], f32, tag="st_ps")
            nc.tensor.transpose(st_ps[:], s_til[:], ident[:])
            stt = sb.tile([NT, 128], f32, tag="stt")
            nc.vector.tensor_copy(stt[:], st_ps[:])
            nc.sync.dma_start(s_dram[:], stt[:])

            # posT + scale-to-f16 streamed per chunk
            for ch in range(8):
                posT_chunk(0, ch)
                posT_chunk(1, ch)
                cs = slice(ch * 512, (ch + 1) * 512)
                sbc = xstage_pool.tile([128, 512], f32, tag="sbc",
                                       name=f"sbc{ch}")
                nc.sync.dma_start(
                    sbc[:], s_dram[:].flatten()[cs].partition_broadcast(128))
                for ct in range(2):
                    nc.vector.tensor_tensor(post16[ct][:, cs],
                                            post_raw[ct][:, cs],
                                            sbc[:], op=Alu.mult)

        # ---------------- steady loop over query tiles ----------------
        e_pool = ctx.enter_context(tc.tile_pool(name="e_sb", bufs=2))
        a_pool = ctx.enter_context(tc.tile_pool(name="a_sb", bufs=2))
        at_pool = ctx.enter_context(tc.tile_pool(name="at_sb", bufs=5))
        osb_pool = ctx.enter_context(tc.tile_pool(name="osb_sb", bufs=2))
        cands_pool = ctx.enter_context(tc.tile_pool(name="cands_sb", bufs=2))
        asum_pool = ctx.enter_context(tc.tile_pool(name="asum_sb", bufs=6))
        ps_sim = ctx.enter_context(tc.tile_pool(name="ps_sim", bufs=5, space="PSUM"))
        ps_oe = ctx.enter_context(tc.tile_pool(name="ps_oe", bufs=2, space="PSUM"))

        PIPE = 3  # gather matmuls lag the sim by 3 tiles to hide selection+DMA

        front = {}   # T -> (E32, cands)
        state = {}   # T -> (AT, asum, oe)

        def emit_front_half(T, half):
            qs = slice(T * 128, (T + 1) * 128)
            if half == 0:
                E32 = e_pool.tile([128, N], f32, tag="E32", name=f"E32_{T}")
                cands = cands_pool.tile([128, 128], f32, tag="cands")
                front[T] = (E32, cands)
            E32, cands = front[T]
            # 2-pass fp16 sim (stationary shared across 4 moving chunks to
            # coalesce weight loads), exp to SBUF, max8 cands per 256 cols
            sms = [ps_sim.tile([128, 512], f32, tag="sm",
                               name=f"sm{T}_{half}_{r}") for r in range(4)]
            for ct in range(2):
                for r in range(4):
                    o = half * 2048 + r * 512
                    nc.tensor.matmul(sms[r][:], post16[ct][:, qs],
                                     post16[ct][:, o:o + 512],
                                     start=(ct == 0), stop=(ct == 1))
            for r in range(4):
                o = half * 2048 + r * 512
                nc.scalar.activation(E32[:, o:o + 512], sms[r][:], AF.Exp)
                if r % 2 == 1:
                    for c in range(4):
                        gc = (half * 2 + r // 2) * 4 + c
                        nc.vector.max(cands[:, gc * 8:(gc + 1) * 8],
                                      E32[:, gc * 256:(gc + 1) * 256])

        def emit_selection(T):
            E32, cands = front.pop(T)
            # exact top-32 threshold in exp space: 4 rounds of max8+replace
            r8 = cands_pool.tile([128, 8], f32, tag="r8")
            for rnd in range(4):
                nc.vector.max(r8[:], cands[:])
                if rnd < 3:
                    nc.vector.match_replace(out=cands[:], in_to_replace=r8[:],
                                            in_values=cands[:], imm_value=NEG)

            # fused mask+weights: A = (E >= v32) * E, denominator via accum
            A = a_pool.tile([128, N], f16, tag="A")
            asum = asum_pool.tile([128, 1], f32, tag="asum")
            nc.vector.scalar_tensor_tensor(A[:], E32[:], r8[:, 7:8], E32[:],
                                           op0=Alu.is_ge, op1=Alu.mult,
                                           accum_out=asum[:])

            # blocked transpose, SBUF -> SBUF (no DRAM bounce)
            AT = at_pool.tile([128, NT, 128], f16, tag="AT", name=f"AT_{T}")
            nc.sync.dma_start_transpose(AT[:], A[:])
            state[T] = (AT, asum)

        def emit_back_half(T, half):
            AT, asum = state[T][:2]
            if half == 0:
                oe = ps_oe.tile([128, C], f32, tag="oe", name=f"oe{T}")
                state[T] = (AT, asum, oe)
            else:
                oe = state[T][2]
            for j in range(half * 16, half * 16 + 16):
                nc.tensor.matmul(oe[:], AT[:, j, :], featx[j][:],
                                 start=(j == 0), stop=(j == NT - 1))

        def emit_back_fin(T):
            AT, asum, oe = state.pop(T)
            rz = asum_pool.tile([128, 1], f32, tag="rz")
            nc.vector.reciprocal(rz[:], asum[:])
            osb = osb_pool.tile([128, C], f32, tag="osb")
            nc.scalar.activation(osb[:], oe[:], AF.Copy, scale=rz[:])
            nc.sync.dma_start(out_p[T * 128:(T + 1) * 128, :], osb[:])

        for rep in range(reps):
            for T in range(NT + PIPE):
                # interleave gather halves of tile T-PIPE into tile T's sim
                # stream so the PE sees one dense instruction sequence
                if T < NT:
                    emit_front_half(T, 0)
                if T >= PIPE:
                    emit_back_half(T - PIPE, 0)
                if T < NT:
                    emit_front_half(T, 1)
                if T >= PIPE:
                    emit_back_half(T - PIPE, 1)
                if T < NT:
                    emit_selection(T)
                if T >= PIPE:
                    emit_back_fin(T - PIPE)

    nc.compile()
    return nc


def kernel(x, W, bias, k):
    from concourse.bass_utils import run_bass_kernel_spmd

    x = np.asarray(x, dtype=np.float32)
    W = np.asarray(W, dtype=np.float32)
    bias = np.asarray(bias, dtype=np.float32)
    assert int(k) == K and x.shape == (B, N, C)

    if "nc" not in _CACHE:
        _CACHE["nc"] = _build()
    nc = _CACHE["nc"]

    wT = np.ascontiguousarray(W.T)                      # [C, 2C]
    bf = bias.reshape(1, 2 * C)
    bp = np.ascontiguousarray(
        bias[C:].reshape(2, 128).T)                     # [128, 2]
    ident = np.eye(128, dtype=np.float32)

    in_maps = []
    for b in range(B):
        xT = np.ascontiguousarray(x[b].T)               # [C, N]
        in_maps.append({"xT": xT, "wT": wT, "bf": bf, "bp": bp, "ident": ident})

    res = run_bass_kernel_spmd(nc, in_maps, list(range(B)))
    out = np.stack([res.results[b]["out"] for b in range(B)], axis=0)
    return out.astype(np.float32)


# revision 20
# speedup vs baseline: 1.2345x; 1.0087x over previous
"""Trainium2 Bass kernel for nn_Beltrami (retrieval_knn).

Per-core (batch-parallel over 8 cores): fc (f32r hi/lo-split matmuls for exact
pos; hi-only for feat and row norms) -> normalize pos -> quantize posT to fp16
-> cosine sim via 2-pass fp16 matmul (sim noise ~2.5e-5; measured end-to-end
rel err 1.27e-2 < 2e-2 gate) -> exp on Act (PSUM->SBUF, E32 f32, doubles as
the PSUM-freeing copy) -> top-32 threshold via 16x max8 over 256-col chunks +
4-round max8/match_replace refine in exp space -> fused (E>=v32)*E mask via
scalar_tensor_tensor with accum_out denominator -> SBUF->SBUF DMA transpose of
A -> A@feat fp16 gather matmul (software-pipelined 3 tiles behind the sim so
the PE stream stays dense) -> softmax-normalized out.
"""
import sys
import numpy as np

sys.path.insert(0, "/opt/trn_rl_repo")

B, N, C, K = 8, 4096, 256, 32
NT = N // 128          # 32 query tiles of 128 rows
NEG = -1.0e30

_CACHE = {}


def _build(reps=1):
    from contextlib import ExitStack
    import concourse.bass as bass
    import concourse.bacc as bacc
    import concourse.tile as tile
    from concourse import mybir

    f32 = mybir.dt.float32
    f32r = mybir.dt.float32r
    f16 = mybir.dt.float16
    AF = mybir.ActivationFunctionType
    Alu = mybir.AluOpType

    nc = bacc.Bacc("TRN2", target_bir_lowering=False, debug=False, num_devices=8)

    xT_in = nc.declare_dram_parameter("xT", [C, N], f32, isOutput=False)
    wT_in = nc.declare_dram_parameter("wT", [C, 2 * C], f32, isOutput=False)
    bf_in = nc.declare_dram_parameter("bf", [1, 2 * C], f32, isOutput=False)
    bp_in = nc.declare_dram_parameter("bp", [128, 2], f32, isOutput=False)
    id_in = nc.declare_dram_parameter("ident", [128, 128], f32, isOutput=False)
    out_p = nc.declare_dram_parameter("out", [N, C], f32, isOutput=True)
    s_dram = nc.dram_tensor("s_scratch", [NT, 128], f32)

    with tile.TileContext(nc) as tc, ExitStack() as ctx:
        # ---------------- persistent pools ----------------
        persist = ctx.enter_context(tc.tile_pool(name="persist", bufs=1))
        featx_pool = ctx.enter_context(tc.tile_pool(name="featx", bufs=NT))

        # fp16 normalized posT, the only sim operand kept resident
        post16 = [persist.tile([128, N], f16, tag=f"post16_{ct}", name=f"post16_{ct}")
                  for ct in range(2)]
        featx = [featx_pool.tile([128, C], f16, tag="featx", name=f"featx{i}") for i in range(NT)]
        nrm2 = persist.tile([128, NT], f32, tag="nrm2")
        s_til = persist.tile([128, NT], f32, tag="s_til")
        xt_r = [persist.tile([128, N], f32r, tag=f"xt_r{ct}", name=f"xt_r{ct}") for ct in range(2)]
        wt_r = [persist.tile([128, C], f32r, tag=f"wt_r{ct}", name=f"wt_r{ct}") for ct in range(2)]
        bf1_r = persist.tile([1, 2 * C], f32r, tag="bf1_r")
        ones1_r = persist.tile([1, 128], f32r, tag="ones1_r")

        # ---------------- startup: fc + normalize ----------------
        with ExitStack() as sctx:
            sb = sctx.enter_context(tc.tile_pool(name="start_sb", bufs=1))
            ps_fc = sctx.enter_context(tc.tile_pool(name="ps_fc", bufs=3, space="PSUM"))
            ps_pp = sctx.enter_context(tc.tile_pool(name="ps_pp", bufs=3, space="PSUM"))
            ps_tp = sctx.enter_context(tc.tile_pool(name="ps_tp", bufs=1, space="PSUM"))

            xstage_pool = sctx.enter_context(tc.tile_pool(name="xstage_pool", bufs=3))
            wt = [sb.tile([128, 2 * C], f32, tag=f"wt{ct}", name=f"wt{ct}") for ct in range(2)]
            xt_lo = [sb.tile([128, N], f32r, tag=f"xt_lo{ct}", name=f"xt_lo{ct}") for ct in range(2)]
            wph = [sb.tile([128, C], f32r, tag=f"wph{ct}", name=f"wph{ct}") for ct in range(2)]
            wpl = [sb.tile([128, C], f32r, tag=f"wpl{ct}", name=f"wpl{ct}") for ct in range(2)]
            bf1 = sb.tile([1, 2 * C], f32, tag="bf1")
            bp = sb.tile([128, 2], f32, tag="bp")
            ident = sb.tile([128, 128], f32, tag="ident")
            ones1 = sb.tile([1, 128], f32, tag="ones1")
            scrap = sb.tile([128, C], f16, tag="scrap")
            post_raw = [sb.tile([128, N], f32, tag=f"post_raw{ct}", name=f"post_raw{ct}") for ct in range(2)]

            for ct in range(2):
                nc.sync.dma_start(wt[ct][:], wT_in[ct * 128:(ct + 1) * 128, :])
            nc.sync.dma_start(bf1[:], bf_in[:])
            nc.sync.dma_start(bp[:], bp_in[:])
            nc.sync.dma_start(ident[:], id_in[:])
            nc.vector.memset(ones1[:], 1.0)
            for ct in range(2):
                nc.vector.tensor_copy(wt_r[ct][:], wt[ct][:, 0:C])
                nc.vector.tensor_copy(wph[ct][:], wt[ct][:, C:2 * C])
                nc.vector.tensor_tensor(wpl[ct][:], wt[ct][:, C:2 * C],
                                        wph[ct][:], op=Alu.subtract)
            nc.vector.tensor_copy(bf1_r[:], bf1[:])
            nc.vector.tensor_copy(ones1_r[:], ones1[:])

            # staging + fc interleaved per 512-chunk so PE stays dense
            def stage_chunk(ch):
                cs = slice(ch * 512, (ch + 1) * 512)
                for ct in range(2):
                    xstage = xstage_pool.tile([128, 512], f32, tag="xstage",
                                              name=f"xstage{ct}_{ch}")
                    nc.sync.dma_start(xstage[:], xT_in[ct * 128:(ct + 1) * 128, cs])
                    nc.vector.tensor_copy(xt_r[ct][:, cs], xstage[:])
                    nc.vector.tensor_tensor(xt_lo[ct][:, cs], xstage[:],
                                            xt_r[ct][:, cs], op=Alu.subtract)

            def posT_chunk(dt, ch):
                pp = ps_pp.tile([128, 512], f32, tag="pp", name=f"pp{dt}_{ch}")
                ds_ = slice(dt * 128, (dt + 1) * 128)
                cs_ = slice(ch * 512, (ch + 1) * 512)
                for ci, (lh, rh) in enumerate(
                        [(wph[0], xt_r[0]), (wph[0], xt_lo[0]), (wpl[0], xt_r[0]),
                         (wph[1], xt_r[1]), (wph[1], xt_lo[1]), (wpl[1], xt_r[1])]):
                    nc.tensor.matmul(pp[:], lh[:, ds_], rh[:, cs_],
                                     start=(ci == 0), stop=(ci == 5))
                nc.scalar.activation(
                    post_raw[dt][:, ch * 512:(ch + 1) * 512], pp[:],
                    AF.Identity, bias=bp[:, dt:dt + 1])

            for ch in range(8):
                stage_chunk(ch)
            for nt in range(NT):
                # hi-only pos fc: nrm2 tolerates ~1e-3 relative error (a
                # norm error scales a whole sim row, never reordering it).
                # feat fc is deferred into the loop's gather-free PE slots.
                fc = ps_fc.tile([128, C], f32, tag="fc")
                ns = slice(nt * 128, (nt + 1) * 128)
                nc.tensor.matmul(fc[:], xt_r[0][:, ns], wph[0][:],
                                 start=True, stop=False)
                nc.tensor.matmul(fc[:], xt_r[1][:, ns], wph[1][:],
                                 start=False, stop=False)
                nc.tensor.matmul(fc[:], ones1_r[:], bf1_r[:, C:2 * C],
                                 start=False, stop=True)
                nc.scalar.activation(scrap[:], fc[:], AF.Square,
                                     accum_out=nrm2[:, nt:nt + 1])

            # rsqrt of norms with two Newton steps (overlaps posT below)
            r0 = sb.tile([128, NT], f32, tag="r0")
            u = sb.tile([128, NT], f32, tag="u")
            nc.vector.reciprocal(r0[:], nrm2[:])
            nc.scalar.activation(s_til[:], r0[:], AF.Sqrt)
            for _ in range(2):
                nc.vector.tensor_tensor(u[:], s_til[:], s_til[:], op=Alu.mult)
                nc.vector.tensor_tensor(u[:], u[:], nrm2[:], op=Alu.mult)
                nc.vector.tensor_scalar(u[:], u[:], -0.5, scalar2=1.5,
                                        op0=Alu.mult, op1=Alu.add)
                nc.vector.tensor_tensor(s_til[:], s_til[:], u[:], op=Alu.mult)

            # transpose s [128, NT] -> [NT, 128], bounce via DRAM, broadcast-load
            st_ps = ps_tp.tile([NT, 128], f32, tag="st_ps")
            nc.tensor.transpose(st_ps[:], s_til[:], ident[:])
            stt = sb.tile([NT, 128], f32, tag="stt")
            nc.vector.tensor_copy(stt[:], st_ps[:])
            nc.sync.dma_start(s_dram[:], stt[:])

            # posT + scale-to-f16 streamed per chunk
            for ch in range(8):
                posT_chunk(0, ch)
                posT_chunk(1, ch)
                cs = slice(ch * 512, (ch + 1) * 512)
                sbc = xstage_pool.tile([128, 512], f32, tag="sbc",
                                       name=f"sbc{ch}")
                nc.sync.dma_start(
                    sbc[:], s_dram[:].flatten()[cs].partition_broadcast(128))
                for ct in range(2):
                    nc.vector.tensor_tensor(post16[ct][:, cs],
                                            post_raw[ct][:, cs],
                                            sbc[:], op=Alu.mult)

        # ---------------- steady loop over query tiles ----------------
        e_pool = ctx.enter_context(tc.tile_pool(name="e_sb", bufs=2))
        a_pool = ctx.enter_context(tc.tile_pool(name="a_sb", bufs=2))
        at_pool = ctx.enter_context(tc.tile_pool(name="at_sb", bufs=5))
        osb_pool = ctx.enter_context(tc.tile_pool(name="osb_sb", bufs=2))
        cands_pool = ctx.enter_context(tc.tile_pool(name="cands_sb", bufs=2))
        asum_pool = ctx.enter_context(tc.tile_pool(name="asum_sb", bufs=6))
        ps_sim = ctx.enter_context(tc.tile_pool(name="ps_sim", bufs=4, space="PSUM"))
        ps_oe = ctx.enter_context(tc.tile_pool(name="ps_oe", bufs=2, space="PSUM"))
        ps_fe = ctx.enter_context(tc.tile_pool(name="ps_fe", bufs=2, space="PSUM"))

        def emit_feat(nt):
            fcf = ps_fe.tile([128, C], f32, tag="fcf", name=f"fcf{nt}")
            ns = slice(nt * 128, (nt + 1) * 128)
            nc.tensor.matmul(fcf[:], xt_r[0][:, ns], wt_r[0][:],
                             start=True, stop=False)
            nc.tensor.matmul(fcf[:], xt_r[1][:, ns], wt_r[1][:],
                             start=False, stop=False)
            nc.tensor.matmul(fcf[:], ones1_r[:], bf1_r[:, 0:C],
                             start=False, stop=True)
            nc.scalar.activation(featx[nt][:], fcf[:], AF.Copy)

        PIPE = 3  # gather matmuls lag the sim by 3 tiles to hide selection+DMA

        front = {}   # T -> (E32, cands)
        state = {}   # T -> (AT, asum, oe)

        def emit_front_half(T, half):
            qs = slice(T * 128, (T + 1) * 128)
            if half == 0:
                E32 = e_pool.tile([128, N], f32, tag="E32", name=f"E32_{T}")
                cands = cands_pool.tile([128, 128], f32, tag="cands")
                front[T] = (E32, cands)
            E32, cands = front[T]
            # 2-pass fp16 sim (stationary shared across 4 moving chunks to
            # coalesce weight loads), exp to SBUF, max8 cands per 256 cols
            sms = [ps_sim.tile([128, 512], f32, tag="sm",
                               name=f"sm{T}_{half}_{r}") for r in range(4)]
            for ct in range(2):
                for r in range(4):
                    o = half * 2048 + r * 512
                    nc.tensor.matmul(sms[r][:], post16[ct][:, qs],
                                     post16[ct][:, o:o + 512],
                                     start=(ct == 0), stop=(ct == 1))
            for r in range(4):
                o = half * 2048 + r * 512
                nc.scalar.activation(E32[:, o:o + 512], sms[r][:], AF.Exp)
                if r % 2 == 1:
                    for c in range(4):
                        gc = (half * 2 + r // 2) * 4 + c
                        nc.vector.max(cands[:, gc * 8:(gc + 1) * 8],
                                      E32[:, gc * 256:(gc + 1) * 256])

        def emit_selection(T):
            E32, cands = front.pop(T)
            # exact top-32 threshold in exp space: 4 rounds of max8+replace
            r8 = cands_pool.tile([128, 8], f32, tag="r8")
            for rnd in range(4):
                nc.vector.max(r8[:], cands[:])
                if rnd < 3:
                    nc.vector.match_replace(out=cands[:], in_to_replace=r8[:],
                                            in_values=cands[:], imm_value=NEG)

            # fused mask+weights: A = (E >= v32) * E, denominator via accum
            A = a_pool.tile([128, N], f16, tag="A")
            asum = asum_pool.tile([128, 1], f32, tag="asum")
            nc.vector.scalar_tensor_tensor(A[:], E32[:], r8[:, 7:8], E32[:],
                                           op0=Alu.is_ge, op1=Alu.mult,
                                           accum_out=asum[:])

            # blocked transpose, SBUF -> SBUF (no DRAM bounce)
            AT = at_pool.tile([128, NT, 128], f16, tag="AT", name=f"AT_{T}")
            nc.sync.dma_start_transpose(AT[:], A[:])
            state[T] = (AT, asum)

        def emit_back_half(T, half):
            AT, asum = state[T][:2]
            if half == 0:
                oe = ps_oe.tile([128, C], f32, tag="oe", name=f"oe{T}")
                state[T] = (AT, asum, oe)
            else:
                oe = state[T][2]
            for j in range(half * 16, half * 16 + 16):
                nc.tensor.matmul(oe[:], AT[:, j, :], featx[j][:],
                                 start=(j == 0), stop=(j == NT - 1))

        def emit_back_fin(T):
            AT, asum, oe = state.pop(T)
            rz = asum_pool.tile([128, 1], f32, tag="rz")
            nc.vector.reciprocal(rz[:], asum[:])
            osb = osb_pool.tile([128, C], f32, tag="osb")
            nc.scalar.activation(osb[:], oe[:], AF.Copy, scale=rz[:])
            nc.sync.dma_start(out_p[T * 128:(T + 1) * 128, :], osb[:])

        for rep in range(reps):
            for T in range(NT + PIPE):
                for k in range(8):
                    nt = T * 8 + k
                    if nt < NT:
                        emit_feat(nt)
                # interleave gather halves of tile T-PIPE into tile T's sim
                # stream so the PE sees one dense instruction sequence
                if T < NT:
                    emit_front_half(T, 0)
                if T >= PIPE:
                    emit_back_half(T - PIPE, 0)
                if T < NT:
                    emit_front_half(T, 1)
                if T >= PIPE:
                    emit_back_half(T - PIPE, 1)
                if T < NT:
                    emit_selection(T)
                if T >= PIPE:
                    emit_back_fin(T - PIPE)

    nc.compile()
    return nc


def kernel(x, W, bias, k):
    from concourse.bass_utils import run_bass_kernel_spmd

    x = np.asarray(x, dtype=np.float32)
    W = np.asarray(W, dtype=np.float32)
    bias = np.asarray(bias, dtype=np.float32)
    assert int(k) == K and x.shape == (B, N, C)

    if "nc" not in _CACHE:
        _CACHE["nc"] = _build()
    nc = _CACHE["nc"]

    wT = np.ascontiguousarray(W.T)                      # [C, 2C]
    bf = bias.reshape(1, 2 * C)
    bp = np.ascontiguousarray(
        bias[C:].reshape(2, 128).T)                     # [128, 2]
    ident = np.eye(128, dtype=np.float32)

    in_maps = []
    for b in range(B):
        xT = np.ascontiguousarray(x[b].T)               # [C, N]
        in_maps.append({"xT": xT, "wT": wT, "bf": bf, "bp": bp, "ident": ident})

    res = run_bass_kernel_spmd(nc, in_maps, list(range(B)))
    out = np.stack([res.results[b]["out"] for b in range(B)], axis=0)
    return out.astype(np.float32)
